# revision 1
# baseline (speedup 1.0000x reference)
"""RNN-T JointNetwork Trainium2 kernel.

logits[b,t,u,v] = sum_j W_out[v,j] * tanh(f[b,t,j] + g[b,u,j]) + b_out[v]
  f = enc_out @ W_enc.T   [B,T,640]
  g = pred_out @ W_pred.T [B,U,640]

Sharding: data-parallel over B=8 across the 8 NeuronCores (1 batch/core).

Per-core device program (everything resident on-chip):
  phase 1: fT = W_enc @ enc.T -> [640,256] f32 accumulated in PSUM (stays
           there; ScalarE reads PSUM faster than SBUF), gT -> [640,64]
           copied to SBUF (activation bias operands must be SBUF).
           Inputs bf16 (host-cast) so phase 1 runs at full PE rate.
  phase 2: per u: combT_u[j,t] = tanh(fT + gT[:,u]) via ScalarE activation
           with per-partition bias (u-major ordering turns the broadcast
           into a partition-axis bias), output cast to bf16
  phase 3: logits rows = combT_u.T @ W_outT in bf16, K=640 as 5x128 chunks
           accumulated into PSUM [128 rows, 512 vocab]
  phase 4: VectorE tensor_add(psum, bias) -> SBUF f32, DMA out with
           per-partition 4KB-contiguous rows.
"""

import sys

for _p in ("/opt/trn_rl_repo",):
    if _p not in sys.path:
        sys.path.insert(0, _p)

import numpy as np
import ml_dtypes

B, T, U = 8, 256, 64
D_ENC, D_PRED, D_JOINT, VOCAB = 512, 512, 640, 1024
KE = D_ENC // 128   # 4 contraction chunks for enc/pred matmuls
KJ = D_JOINT // 128  # 5 contraction chunks for the vocab matmul
N_CORES = 8

_compiled = None


def _build():
    import concourse.bacc as bacc
    import concourse.bass as bass
    import concourse.mybir as mybir
    import concourse.tile as tile

    f32 = mybir.dt.float32
    bf16 = mybir.dt.bfloat16
    PSUM = bass.MemorySpace.PSUM
    tanh = mybir.ActivationFunctionType.Tanh

    nc = bacc.Bacc(
        "TRN2",
        target_bir_lowering=False,
        debug=False,
        enable_asserts=False,
    )

    enc_d = nc.dram_tensor("enc", [128, KE, T], bf16, kind="ExternalInput")
    pred_d = nc.dram_tensor("pred", [128, KE, U], bf16, kind="ExternalInput")
    wenc_d = nc.dram_tensor("wenc", [128, KE, D_JOINT], bf16, kind="ExternalInput")
    wpred_d = nc.dram_tensor("wpred", [128, KE, D_JOINT], bf16, kind="ExternalInput")
    wout_d = nc.dram_tensor("wout", [128, KJ, VOCAB], bf16, kind="ExternalInput")
    bias_d = nc.dram_tensor("bias", [1, VOCAB], f32, kind="ExternalInput")
    out_d = nc.dram_tensor("out", [T, U, VOCAB], f32, kind="ExternalOutput")

    with tile.TileContext(nc) as tc:
        with (
            tc.tile_pool(name="const", bufs=1) as const,
            tc.tile_pool(name="comb", bufs=3) as comb_pool,
            tc.tile_pool(name="outsb", bufs=4) as out_pool,
            tc.tile_pool(name="psf", bufs=1, space=PSUM) as psf,
        ):
            # Trigger the Tanh ACT table load before any data arrives.
            warm = const.tile([1, 8], f32)
            warm2 = const.tile([1, 8], f32)
            nc.vector.memset(warm[:], 0.0)
            nc.scalar.activation(warm2[:], warm[:], tanh)

            pred_sb = const.tile([128, KE, U], bf16)
            wpred_sb = const.tile([128, KE, D_JOINT], bf16)
            enc_sb = const.tile([128, KE, T], bf16)
            wenc_sb = const.tile([128, KE, D_JOINT], bf16)
            wout_sb = const.tile([128, KJ, VOCAB], bf16)
            bias_row = const.tile([1, VOCAB], f32)
            bias_sb = const.tile([128, VOCAB], f32)
            ones_sb = const.tile([1, 128], f32)
            gT_sb = const.tile([128, KJ, U], f32)
            fT_ps = psf.tile([128, KJ, T], f32)  # 5 KiB/partition -> 3 banks

            # PE warmup: dummy matmuls on zeroed data while input DMAs are
            # in flight, so HAM un-throttles before the real matmuls start.
            wz = const.tile([128, 512], bf16)
            nc.vector.memset(wz[:], 0.0)
            nc.vector.memset(ones_sb[:], 1.0)

            # Input DMA triggers spread across the three DMA-capable
            # engines so they issue in parallel.
            nc.sync.dma_start(pred_sb[:], pred_d[:])
            nc.gpsimd.dma_start(wpred_sb[:], wpred_d[:])
            nc.scalar.dma_start(enc_sb[:], enc_d[:])
            nc.sync.dma_start(wenc_sb[:], wenc_d[:])
            nc.gpsimd.dma_start(wout_sb[:], wout_d[:])
            nc.scalar.dma_start(bias_row[:], bias_d[:])

            with tc.tile_pool(name="psw", bufs=1, space=PSUM) as psw:
                pw = psw.tile([128, 512], f32)
                for i in range(10):
                    nc.tensor.matmul(pw[:], wz[:, :128], wz[:], start=True, stop=True)

            # phase 1: j-outer accumulation groups (a group must fully
            # close before another start=True touches its PSUM bank);
            # gT copies interleave under the following fT matmul group.
            with tc.tile_pool(name="psg", bufs=2, space=PSUM) as psg:
                for j in range(KJ):
                    ps = psg.tile([128, U], f32, tag="psg")
                    for k in range(KE):
                        nc.tensor.matmul(
                            ps[:],
                            wpred_sb[:, k, j * 128:(j + 1) * 128],
                            pred_sb[:, k, :],
                            start=(k == 0),
                            stop=(k == KE - 1),
                        )
                    nc.scalar.copy(gT_sb[:, j, :], ps[:])
                    for k in range(KE):
                        nc.tensor.matmul(
                            fT_ps[:, j, :],
                            wenc_sb[:, k, j * 128:(j + 1) * 128],
                            enc_sb[:, k, :],
                            start=(k == 0),
                            stop=(k == KE - 1),
                        )

                # replicate b_out across partitions with two rank-1 matmuls
                bps = psg.tile([128, 512], f32, tag="psg", name="bps")
                nc.tensor.matmul(bps[:], ones_sb[:], bias_row[:, 0:512],
                                 start=True, stop=True)
                nc.vector.tensor_copy(bias_sb[:, 0:512], bps[:])
                bps2 = psg.tile([128, 512], f32, tag="psg", name="bps2")
                nc.tensor.matmul(bps2[:], ones_sb[:], bias_row[:, 512:1024],
                                 start=True, stop=True)
                nc.vector.tensor_copy(bias_sb[:, 512:1024], bps2[:])

            with tc.tile_pool(name="pso", bufs=5, space=PSUM) as pso:
                for u in range(U):
                    comb = comb_pool.tile([128, KJ, T], bf16, tag="comb")
                    for j in range(KJ):
                        nc.scalar.activation(
                            comb[:, j, :],
                            fT_ps[:, j, :],
                            tanh,
                            bias=gT_sb[:, j, u:u + 1],
                        )
                    for rt in range(T // 128):
                        rows = slice(rt * 128, (rt + 1) * 128)
                        last = (u == U - 1 and rt == T // 128 - 1)
                        po0 = pso.tile([128, 512], f32, tag="pso")
                        po1 = pso.tile([128, 512], f32, tag="pso")
                        ob = out_pool.tile([128, VOCAB], f32, tag="ob")
                        if not last:
                            for j in range(KJ):
                                lhsT = comb[:, j, rows]
                                nc.tensor.matmul(
                                    po0[:], lhsT, wout_sb[:, j, 0:512],
                                    start=(j == 0), stop=(j == KJ - 1),
                                )
                                nc.tensor.matmul(
                                    po1[:], lhsT, wout_sb[:, j, 512:1024],
                                    start=(j == 0), stop=(j == KJ - 1),
                                )
                            nc.vector.tensor_add(ob[:, 0:512], po0[:], bias_sb[:, 0:512])
                            nc.vector.tensor_add(ob[:, 512:1024], po1[:], bias_sb[:, 512:1024])
                            nc.sync.dma_start(out_d[rows, u, :], ob[:])
                        else:
                            # final tile: close the two PSUM groups one after
                            # the other and stream out in quarters so the
                            # epilogue after the very last matmul is short
                            for j in range(KJ):
                                nc.tensor.matmul(
                                    po0[:], comb[:, j, rows], wout_sb[:, j, 0:512],
                                    start=(j == 0), stop=(j == KJ - 1),
                                )
                            nc.vector.tensor_add(ob[:, 0:512], po0[:], bias_sb[:, 0:512])
                            nc.sync.dma_start(out_d[rows, u, 0:512], ob[:, 0:512])
                            for j in range(KJ):
                                nc.tensor.matmul(
                                    po1[:], comb[:, j, rows], wout_sb[:, j, 512:1024],
                                    start=(j == 0), stop=(j == KJ - 1),
                                )
                            nc.vector.tensor_add(ob[:, 512:768], po1[:, 0:256], bias_sb[:, 512:768])
                            nc.gpsimd.dma_start(out_d[rows, u, 512:768], ob[:, 512:768])
                            nc.vector.tensor_add(ob[:, 768:1024], po1[:, 256:512], bias_sb[:, 768:1024])
                            nc.scalar.dma_start(out_d[rows, u, 768:1024], ob[:, 768:1024])

    nc.compile()
    return nc


def _get_compiled():
    global _compiled
    if _compiled is None:
        _compiled = _build()
    return _compiled


def _prep_inputs(enc_out, pred_out, W_enc, W_pred, W_out, b_out):
    bf = ml_dtypes.bfloat16
    enc_out = np.asarray(enc_out, dtype=np.float32)
    pred_out = np.asarray(pred_out, dtype=np.float32)
    W_enc = np.asarray(W_enc, dtype=np.float32)
    W_pred = np.asarray(W_pred, dtype=np.float32)
    W_out = np.asarray(W_out, dtype=np.float32)
    b_out = np.asarray(b_out, dtype=np.float32)

    # [d, x] -> [128, d//128, x]: partition-major chunking of the d axis
    wenc = np.ascontiguousarray(
        W_enc.T.reshape(KE, 128, D_JOINT).transpose(1, 0, 2)).astype(bf)
    wpred = np.ascontiguousarray(
        W_pred.T.reshape(KE, 128, D_JOINT).transpose(1, 0, 2)).astype(bf)
    wout = np.ascontiguousarray(
        W_out.T.reshape(KJ, 128, VOCAB).transpose(1, 0, 2)).astype(bf)
    bias = np.ascontiguousarray(b_out.reshape(1, VOCAB))

    in_maps = []
    for b in range(B):
        encb = np.ascontiguousarray(
            enc_out[b].T.reshape(KE, 128, T).transpose(1, 0, 2)).astype(bf)
        predb = np.ascontiguousarray(
            pred_out[b].T.reshape(KE, 128, U).transpose(1, 0, 2)).astype(bf)
        in_maps.append({
            "enc": encb, "pred": predb, "wenc": wenc, "wpred": wpred,
            "wout": wout, "bias": bias,
        })
    return in_maps


def run(inputs, trace=False, **kwargs):
    from concourse.bass_utils import run_bass_kernel_spmd

    nc = _get_compiled()
    in_maps = _prep_inputs(**inputs)
    res = run_bass_kernel_spmd(
        nc, in_maps, core_ids=list(range(N_CORES)), trace=trace, **kwargs)
    out = np.stack([res.results[b]["out"] for b in range(B)], axis=0)
    return out.astype(np.float32, copy=False), res


def kernel(**inputs):
    out, _ = run(inputs, trace=False)
    return out

